# revision 34
# baseline (speedup 1.0000x reference)
"""K-center style kernel: argmax_i min_j ||A_i - B_j|| on 8 NeuronCores.

Strategy:
  - Host prefilter (sound): ub_i = na_i + min_{j in S}(nb_j - 2 a_i.b_j) + pad
    is a true upper bound on d^2_min(i) for any probe subset S (|S|=512).
    v0 = exact d^2_min (float64) of the best-ub row is a lower bound on the
    final max. Rows with ub_i < v0 cannot be the argmax and are dropped
    before touching the device (~98% of rows on randn inputs).
  - Hybrid shard over 8 cores: 4 row-groups x 2 column-groups. Core (r,g)
    gets survivor quarter r (padded to 128*T rows) and half g of B's
    columns (2560, sorted by nb = ||b||^2, B padded to 5120). Host takes
    the min over g and combines.
  - Per core: fp8 DoubleRow matmuls produce -2 a_i.b_j in PSUM; grouped
    DVE tensor_reduce(min) per PSUM tile gives per-group minima (groups of
    128 nb-sorted columns, nb midpoint added afterwards); per row-tile a
    tiny add(nb_mid) + min-reduce yields m[i] ~= min_j (nb_j - 2 a_i.b_j).
  - All device input (A-tiles, B^T, nb group mids as raw bytes) is packed
    into one fp8 DRAM tensor so every DMA has >=2KB-per-partition
    descriptors, split across the sync+scalar HWDGE queues.
  - Host: D_approx = sqrt(max(na + m, 0)) over survivors; select candidates
    within DELTA of the max; rescore candidates exactly in float64; return
    (argmax int32, max float32).

The host rescore makes the final answer exact regardless of device
precision; the device pass only needs the true argmax inside the
candidate set. Device error sources: fp8 input rounding + nb grouping,
both ~1e-2 in D units. DELTA = 1.0 is far above both.
"""

import numpy as np
import ml_dtypes

N_CORES = 8
ROW_GROUPS = 4
COL_GROUPS = 2
M_B = 5000
M_PAD = 5120                              # padded B columns (sorted by nb)
M_CORE = M_PAD // COL_GROUPS              # 2560 columns per core
D_FEAT = 512
N_CHUNK = 512                             # matmul free dim = one fp32 PSUM bank
GRP = 128                                 # B columns per min-group (sorted by nb)
N_GROUPS = M_CORE // GRP                  # 20
N_PROBE = 512                             # host prefilter probe columns

DELTA = 1.0  # candidate slack in D units (covers fp8 e4m3 + grouping error)

_compiled = {}


def build_program(row_tiles, m_b=M_CORE, n_chunk=N_CHUNK, grp=GRP):
    import concourse.tile as tile
    import concourse.mybir as mybir
    from concourse import bacc

    n_chunks = m_b // n_chunk            # 5
    n_groups = m_b // grp                # 20
    gpc = n_chunk // grp                 # groups per chunk
    assert m_b % n_chunk == 0 and n_chunk % grp == 0

    AT = 512 * row_tiles                 # A region bytes per partition
    L = AT + 4 * m_b + 4 * n_groups      # + B^T region + nbg raw bytes

    nc = bacc.Bacc("TRN2", target_bir_lowering=False, debug=False)
    inb = nc.dram_tensor(
        "INB", [128, L], mybir.dt.float8e4, kind="ExternalInput"
    ).ap()
    mout = nc.dram_tensor(
        "M", [128, row_tiles], mybir.dt.float32, kind="ExternalOutput"
    ).ap()

    fp32 = mybir.dt.float32
    fp8 = mybir.dt.float8e4
    DR = mybir.MatmulPerfMode.DoubleRow
    add = mybir.AluOpType.add
    amin = mybir.AluOpType.min
    X = mybir.AxisListType.X

    # chunk groups -> one PSUM tile each; last group small so the final
    # DVE reduce (tail latency after the last matmul) is short
    psgroups = []
    c = 0
    while c < n_chunks:
        w = min(4, n_chunks - c)
        psgroups.append((c, w))
        c += w

    def bofs(kt, half, c0):
        return AT + kt * 2 * m_b + half * m_b + c0 * n_chunk

    with tile.TileContext(nc) as tc:
        with (
            tc.tile_pool(name="const", bufs=1) as cpool,
            tc.tile_pool(name="psum", bufs=2, space="PSUM") as pspool,
            tc.tile_pool(name="work", bufs=2 * row_tiles + 2) as gmpool,
        ):
            spool = mpool = gmpool
            # PE p-state warmup: the PE only reaches full clock after ~3us
            # of continuous execution, and it would otherwise sit idle while
            # the input DMAs land. Stream dummy matmuls (memset SBUF ->
            # sacrificial PSUM bank) from t~6.5us so the real matmuls issue
            # against a hot array.
            dummy_sb = cpool.tile([128, 1024], fp8)
            nc.gpsimd.memset(dummy_sb[:], 0)
            first_ps = pspool.tile([128, 4 * n_chunk], fp32, tag="ps")
            for _ in range(11):
                nc.tensor.matmul(
                    first_ps[:, 0:n_chunk],
                    lhsT=dummy_sb[:, 0:256].rearrange("p (two f) -> p two f", two=2),
                    rhs=dummy_sb[:].rearrange("p (two j) -> p two j", two=2),
                    start=True,
                    stop=True,
                    perf_mode=DR,
                )

            # One resident SBUF tile holds A-tiles | B^T | nbg bytes, loaded
            # by 8 DMAs (2KB+ descriptors) alternating sync/scalar queues,
            # first-psum-group slices first so matmuls unblock earliest.
            inb_sb = cpool.tile([128, L], fp8)
            c0, w0 = psgroups[0]
            nc.sync.dma_start(
                out=inb_sb[:, 0 : AT + w0 * n_chunk],
                in_=inb[:, 0 : AT + w0 * n_chunk],
            )
            nc.scalar.dma_start(
                out=inb_sb[:, bofs(0, 1, 0) : bofs(0, 1, 0) + w0 * n_chunk],
                in_=inb[:, bofs(0, 1, 0) : bofs(0, 1, 0) + w0 * n_chunk],
            )
            nc.sync.dma_start(
                out=inb_sb[:, bofs(1, 0, 0) : bofs(1, 0, 0) + w0 * n_chunk],
                in_=inb[:, bofs(1, 0, 0) : bofs(1, 0, 0) + w0 * n_chunk],
            )
            nc.scalar.dma_start(
                out=inb_sb[:, bofs(1, 1, 0) : bofs(1, 1, 0) + w0 * n_chunk],
                in_=inb[:, bofs(1, 1, 0) : bofs(1, 1, 0) + w0 * n_chunk],
            )
            qflip = 0
            for c1, w1 in psgroups[1:]:
                for kt in range(2):
                    for half in range(2):
                        lo = bofs(kt, half, c1)
                        hi = lo + w1 * n_chunk
                        if kt == 1 and half == 1 and (c1, w1) == psgroups[-1]:
                            hi = L  # append nbg bytes to the last B slice
                        eng = nc.sync if qflip % 2 == 0 else nc.scalar
                        qflip += 1
                        eng.dma_start(out=inb_sb[:, lo:hi], in_=inb[:, lo:hi])

            a_all = inb_sb[:, 0:AT]
            nbg_sb = inb_sb[:, AT + 4 * m_b : L].bitcast(fp32)  # [128, n_groups]
            m_sb = mpool.tile([128, row_tiles], fp32)

            gm_tiles = [
                gmpool.tile([128, n_groups], fp32, tag="gm", name=f"gm{i}")
                for i in range(row_tiles)
            ]
            last_c0 = psgroups[-1][0]
            for c0, w in psgroups:
                for it in range(row_tiles):
                    if c0 == 0 and it == 0:
                        ps = first_ps
                    else:
                        ps = pspool.tile([128, 4 * n_chunk], fp32, tag="ps")
                    for nl in range(w):
                        n = c0 + nl
                        for kt in range(2):
                            lhsT3 = a_all[
                                :, it * 512 + kt * 256 : it * 512 + (kt + 1) * 256
                            ].rearrange("p (two f) -> p two f", two=2)
                            # rhs: [p, two(half), j] strided view over inb_sb
                            rhs = inb_sb[:, bofs(kt, 0, 0) : bofs(kt, 0, 0) + 2 * m_b]
                            rhs = rhs.rearrange("p (two j) -> p two j", two=2)
                            nc.tensor.matmul(
                                ps[:, nl * n_chunk : (nl + 1) * n_chunk],
                                lhsT=lhsT3,
                                rhs=rhs[:, :, n * n_chunk : (n + 1) * n_chunk],
                                start=(kt == 0),
                                stop=(kt == 1),
                                perf_mode=DR,
                            )
                    # split wide groups into halves so the reduce of the
                    # first banks overlaps the matmuls of the last banks
                    hw_ = (w + 1) // 2
                    for h0 in range(0, w, hw_):
                        hn = min(hw_, w - h0)
                        nc.vector.tensor_reduce(
                            out=gm_tiles[it][
                                :, (c0 + h0) * gpc : (c0 + h0 + hn) * gpc
                            ],
                            in_=ps[
                                :, h0 * n_chunk : (h0 + hn) * n_chunk
                            ].rearrange("p (a b) -> p a b", b=grp),
                            axis=X,
                            op=amin,
                        )
                    if c0 == last_c0:
                        s_sb = spool.tile([128, n_groups], fp32)
                        nc.vector.tensor_tensor(
                            out=s_sb[:], in0=gm_tiles[it][:], in1=nbg_sb, op=add
                        )
                        nc.vector.tensor_reduce(
                            out=m_sb[:, it : it + 1], in_=s_sb[:], axis=X, op=amin
                        )
            nc.sync.dma_start(out=mout[:], in_=m_sb[:])
    nc.compile()
    return nc


def prep_inputs(A_sel, B, row_tiles):
    """A_sel: [ROW_GROUPS*128*row_tiles, 512] f32 (padded), B: [M, 512] f32.
    Returns inb [N_CORES, 128, L] fp8 bytes (core = r*COL_GROUPS + g)."""
    e4 = ml_dtypes.float8_e4m3
    B32 = B.astype(np.float32)
    nb32 = (B32**2).sum(axis=1)
    # pad B with copies of column 0 (distance contributions duplicate, min unchanged)
    Bp = np.concatenate([B32, np.broadcast_to(B32[0:1], (M_PAD - M_B, D_FEAT))], axis=0)
    nbp = np.concatenate([nb32, np.broadcast_to(nb32[0:1], (M_PAD - M_B,))])
    order = np.argsort(nbp, kind="stable")
    Bs = Bp[order]
    nbs = nbp[order]

    n_per_rg = 128 * row_tiles
    AT = 512 * row_tiles
    L = AT + 4 * M_CORE + 4 * N_GROUPS

    # A region: [rg, 128p(feat%128), (tile, kt, half, 128i)] of -2A
    Am2 = (-2.0 * A_sel.astype(np.float32)).reshape(ROW_GROUPS, n_per_rg, D_FEAT)
    # feature index = kt*256 + half*128 + p
    atb = np.ascontiguousarray(
        Am2.reshape(ROW_GROUPS, row_tiles, 128, 2, 2, 128).transpose(0, 5, 1, 3, 4, 2)
    ).reshape(ROW_GROUPS, 128, AT).astype(e4)

    inb = np.empty((N_CORES, 128, L), np.uint8)
    for g in range(COL_GROUPS):
        Bg = Bs[g * M_CORE : (g + 1) * M_CORE]
        nbg_g = nbs[g * M_CORE : (g + 1) * M_CORE]
        # B region: [128p, kt(2), half(2), j] = Bg[j, kt*256+half*128+p]
        btb = np.ascontiguousarray(
            Bg.reshape(M_CORE, 2, 2, 128).transpose(3, 1, 2, 0)
        ).reshape(128, 4 * M_CORE).astype(e4)
        # per-group nb midpoint, broadcast to partitions, raw bytes
        gg = nbg_g.reshape(N_GROUPS, GRP)
        nb_mid = ((gg.min(axis=1) + gg.max(axis=1)) * 0.5).astype(np.float32)
        nbg_bytes = np.broadcast_to(
            nb_mid[None, :], (128, N_GROUPS)
        ).astype(np.float32).copy().view(np.uint8).reshape(128, 4 * N_GROUPS)
        for r in range(ROW_GROUPS):
            core = r * COL_GROUPS + g
            inb[core, :, 0:AT] = atb[r].view(np.uint8)
            inb[core, :, AT : AT + 4 * M_CORE] = btb.view(np.uint8)
            inb[core, :, AT + 4 * M_CORE : L] = nbg_bytes
    return inb.view(e4)


def _host_filter(A, B):
    """Sound row prefilter. Returns (survivor_indices, v0).

    ub_i = na_i + min_{j in S}(nb_j - 2 a_i.b_j) + pad >= d^2_min(i) for any
    probe subset S; pad absorbs fp32 matmul rounding. v0 = exact float64
    d^2_min of the best-ub row <= the true max. Rows with ub_i < v0 cannot
    be the argmax.
    """
    na = (A.astype(np.float64) ** 2).sum(axis=1)
    rng = np.random.default_rng(0)
    sel = rng.choice(B.shape[0], N_PROBE, replace=False)
    Bs = np.ascontiguousarray(B[sel]).astype(np.float32)
    nbs = (Bs.astype(np.float64) ** 2).sum(axis=1).astype(np.float32)
    G = np.ascontiguousarray(A.astype(np.float32)) @ Bs.T
    term = (nbs[None, :] - 2.0 * G).min(axis=1).astype(np.float64)
    ub = na + term + 1.0
    k = int(np.argmax(ub))
    B64 = B.astype(np.float64)
    d2k = na[k] + ((B64**2).sum(axis=1) - 2.0 * (B64 @ A[k].astype(np.float64)))
    v0 = float(d2k.min())
    surv = np.where(ub >= v0)[0]
    return surv, v0


def _exact_rescore(A, B, cand):
    A64 = A[cand].astype(np.float64)
    B64 = B.astype(np.float64)
    na = (A64 * A64).sum(axis=1)[:, None]
    nb = (B64 * B64).sum(axis=1)[None, :]
    sq = na - 2.0 * (A64 @ B64.T) + nb
    d = np.sqrt(np.maximum(sq, 0.0))
    return d.min(axis=1)


def _get_compiled(row_tiles):
    if row_tiles not in _compiled:
        _compiled[row_tiles] = build_program(row_tiles)
    return _compiled[row_tiles]


def kernel(A, B, _trace=False):
    from concourse.bass_utils import run_bass_kernel_spmd

    A = np.asarray(A, np.float32)
    B = np.asarray(B, np.float32)

    surv, _v0 = _host_filter(A, B)
    R = len(surv)
    tile_opts = [1, 2, 4, 8]
    T = next((t for t in tile_opts if t * 128 * ROW_GROUPS >= R), None)

    if T is None:
        # Overflow fallback (should not happen for randn inputs): exact
        # host rescore of every survivor, no device pass.
        d_exact = _exact_rescore(A, B, surv)
        w = int(np.argmax(d_exact))
        out = (np.array(int(surv[w]), dtype=np.int32),
               np.array(float(d_exact[w]), dtype=np.float32))
        if _trace:
            return out, None
        return out

    n_rows = T * 128 * ROW_GROUPS
    surv_pad = np.concatenate([surv, np.full(n_rows - R, surv[0], dtype=surv.dtype)])
    A_sel = A[surv_pad]

    inb = prep_inputs(A_sel, B, T)
    nc = _get_compiled(T)
    in_maps = [{"INB": inb[c]} for c in range(N_CORES)]
    res = run_bass_kernel_spmd(nc, in_maps, list(range(N_CORES)), trace=_trace)

    # Gather per-core m: core (r,g) -> [128, T]; combine min over g, then
    # undo the [128, T] (p, it) layout -> row it*128+p within row-group r.
    m_parts = []
    for r in range(ROW_GROUPS):
        mg = np.stack(
            [res.results[r * COL_GROUPS + g]["M"] for g in range(COL_GROUPS)]
        ).min(axis=0)  # [128, T]
        m_parts.append(mg.T.reshape(-1))
    m = np.concatenate(m_parts)
    na = (A_sel.astype(np.float64) ** 2).sum(axis=1)
    d_approx = np.sqrt(np.maximum(na + m, 0.0))
    v = d_approx.max()
    cand_local = np.where(d_approx >= v - DELTA)[0]
    cand = np.unique(surv_pad[cand_local])
    d_exact = _exact_rescore(A, B, cand)
    w = int(np.argmax(d_exact))
    idx = int(cand[w])
    val = float(d_exact[w])
    out = (np.array(idx, dtype=np.int32), np.array(val, dtype=np.float32))
    if _trace:
        return out, res
    return out


# revision 36
# speedup vs baseline: 1.0020x; 1.0020x over previous
"""K-center style kernel: argmax_i min_j ||A_i - B_j|| on 8 NeuronCores.

Strategy:
  - Host prefilter (sound): ub_i = na_i + min_{j in S}(nb_j - 2 a_i.b_j) + pad
    is a true upper bound on d^2_min(i) for any probe subset S (|S|=512).
    v0 = exact d^2_min (float64) of the best-ub row is a lower bound on the
    final max. Rows with ub_i < v0 cannot be the argmax and are dropped
    before touching the device (~98% of rows on randn inputs).
  - Hybrid shard over 8 cores: 4 row-groups x 2 column-groups. Core (r,g)
    gets survivor quarter r (padded to 128*T rows) and half g of B's
    columns (2560, sorted by nb = ||b||^2, B padded to 5120). Host takes
    the min over g and combines.
  - Per core: fp8 DoubleRow matmuls produce -2 a_i.b_j in PSUM; grouped
    DVE tensor_reduce(min) per PSUM tile gives per-group minima (groups of
    128 nb-sorted columns, nb midpoint added afterwards); per row-tile a
    tiny add(nb_mid) + min-reduce yields m[i] ~= min_j (nb_j - 2 a_i.b_j).
  - All device input (A-tiles, B^T, nb group mids as raw bytes) is packed
    into one fp8 DRAM tensor so every DMA has >=2KB-per-partition
    descriptors, split across the sync+scalar HWDGE queues.
  - Host: D_approx = sqrt(max(na + m, 0)) over survivors; select candidates
    within DELTA of the max; rescore candidates exactly in float64; return
    (argmax int32, max float32).

The host rescore makes the final answer exact regardless of device
precision; the device pass only needs the true argmax inside the
candidate set. Device error sources: fp8 input rounding + nb grouping,
both ~1e-2 in D units. DELTA = 1.0 is far above both.
"""

import numpy as np
import ml_dtypes

N_CORES = 8
ROW_GROUPS = 4
COL_GROUPS = 2
M_B = 5000
M_PAD = 5120                              # padded B columns (sorted by nb)
M_CORE = M_PAD // COL_GROUPS              # 2560 columns per core
D_FEAT = 512
N_CHUNK = 512                             # matmul free dim = one fp32 PSUM bank
GRP = 128                                 # B columns per min-group (sorted by nb)
N_GROUPS = M_CORE // GRP                  # 20
N_PROBE = 512                             # host prefilter probe columns

DELTA = 1.0  # candidate slack in D units (covers fp8 e4m3 + grouping error)

_compiled = {}


def build_program(row_tiles, m_b=M_CORE, n_chunk=N_CHUNK, grp=GRP):
    import concourse.tile as tile
    import concourse.mybir as mybir
    from concourse import bacc

    n_chunks = m_b // n_chunk            # 5
    n_groups = m_b // grp                # 20
    gpc = n_chunk // grp                 # groups per chunk
    assert m_b % n_chunk == 0 and n_chunk % grp == 0

    AT = 512 * row_tiles                 # A region bytes per partition
    L = AT + 4 * m_b + 4 * n_groups      # + B^T region + nbg raw bytes

    nc = bacc.Bacc("TRN2", target_bir_lowering=False, debug=False)
    inb = nc.dram_tensor(
        "INB", [128, L], mybir.dt.float8e4, kind="ExternalInput"
    ).ap()
    mout = nc.dram_tensor(
        "M", [128, row_tiles], mybir.dt.float32, kind="ExternalOutput"
    ).ap()

    fp32 = mybir.dt.float32
    fp8 = mybir.dt.float8e4
    DR = mybir.MatmulPerfMode.DoubleRow
    add = mybir.AluOpType.add
    amin = mybir.AluOpType.min
    X = mybir.AxisListType.X

    # chunk groups -> one PSUM tile each; last group small so the final
    # DVE reduce (tail latency after the last matmul) is short
    psgroups = []
    c = 0
    while c < n_chunks:
        w = min(4, n_chunks - c)
        psgroups.append((c, w))
        c += w

    def bofs(kt, half, c0):
        return AT + kt * 2 * m_b + half * m_b + c0 * n_chunk

    with tile.TileContext(nc) as tc:
        with (
            tc.tile_pool(name="const", bufs=1) as cpool,
            tc.tile_pool(name="psum", bufs=2, space="PSUM") as pspool,
            tc.tile_pool(name="work", bufs=2 * row_tiles + 2) as gmpool,
        ):
            spool = mpool = gmpool
            # PE p-state warmup: the PE only reaches full clock after ~3us
            # of continuous execution, and it would otherwise sit idle while
            # the input DMAs land. Stream dummy matmuls (memset SBUF ->
            # sacrificial PSUM bank) from t~6.5us so the real matmuls issue
            # against a hot array.
            dummy_sb = cpool.tile([128, 1024], fp8)
            nc.gpsimd.memset(dummy_sb[:], 0)
            first_ps = pspool.tile([128, 4 * n_chunk], fp32, tag="ps")
            for _ in range(9):
                nc.tensor.matmul(
                    first_ps[:, 0:n_chunk],
                    lhsT=dummy_sb[:, 0:256].rearrange("p (two f) -> p two f", two=2),
                    rhs=dummy_sb[:].rearrange("p (two j) -> p two j", two=2),
                    start=True,
                    stop=True,
                    perf_mode=DR,
                )

            # One resident SBUF tile holds A-tiles | B^T | nbg bytes, loaded
            # by 8 DMAs (2KB+ descriptors) alternating sync/scalar queues,
            # first-psum-group slices first so matmuls unblock earliest.
            inb_sb = cpool.tile([128, L], fp8)
            c0, w0 = psgroups[0]
            nc.sync.dma_start(
                out=inb_sb[:, 0 : AT + w0 * n_chunk],
                in_=inb[:, 0 : AT + w0 * n_chunk],
            )
            nc.scalar.dma_start(
                out=inb_sb[:, bofs(0, 1, 0) : bofs(0, 1, 0) + w0 * n_chunk],
                in_=inb[:, bofs(0, 1, 0) : bofs(0, 1, 0) + w0 * n_chunk],
            )
            nc.sync.dma_start(
                out=inb_sb[:, bofs(1, 0, 0) : bofs(1, 0, 0) + w0 * n_chunk],
                in_=inb[:, bofs(1, 0, 0) : bofs(1, 0, 0) + w0 * n_chunk],
            )
            nc.scalar.dma_start(
                out=inb_sb[:, bofs(1, 1, 0) : bofs(1, 1, 0) + w0 * n_chunk],
                in_=inb[:, bofs(1, 1, 0) : bofs(1, 1, 0) + w0 * n_chunk],
            )
            qflip = 0
            for c1, w1 in psgroups[1:]:
                for kt in range(2):
                    for half in range(2):
                        lo = bofs(kt, half, c1)
                        hi = lo + w1 * n_chunk
                        if kt == 1 and half == 1 and (c1, w1) == psgroups[-1]:
                            hi = L  # append nbg bytes to the last B slice
                        eng = nc.sync if qflip % 2 == 0 else nc.scalar
                        qflip += 1
                        eng.dma_start(out=inb_sb[:, lo:hi], in_=inb[:, lo:hi])

            a_all = inb_sb[:, 0:AT]
            nbg_sb = inb_sb[:, AT + 4 * m_b : L].bitcast(fp32)  # [128, n_groups]
            m_sb = mpool.tile([128, row_tiles], fp32)

            gm_tiles = [
                gmpool.tile([128, n_groups], fp32, tag="gm", name=f"gm{i}")
                for i in range(row_tiles)
            ]
            last_c0 = psgroups[-1][0]
            for c0, w in psgroups:
                for it in range(row_tiles):
                    if c0 == 0 and it == 0:
                        ps = first_ps
                    else:
                        ps = pspool.tile([128, 4 * n_chunk], fp32, tag="ps")
                    for nl in range(w):
                        n = c0 + nl
                        for kt in range(2):
                            lhsT3 = a_all[
                                :, it * 512 + kt * 256 : it * 512 + (kt + 1) * 256
                            ].rearrange("p (two f) -> p two f", two=2)
                            # rhs: [p, two(half), j] strided view over inb_sb
                            rhs = inb_sb[:, bofs(kt, 0, 0) : bofs(kt, 0, 0) + 2 * m_b]
                            rhs = rhs.rearrange("p (two j) -> p two j", two=2)
                            nc.tensor.matmul(
                                ps[:, nl * n_chunk : (nl + 1) * n_chunk],
                                lhsT=lhsT3,
                                rhs=rhs[:, :, n * n_chunk : (n + 1) * n_chunk],
                                start=(kt == 0),
                                stop=(kt == 1),
                                perf_mode=DR,
                            )
                    nc.vector.tensor_reduce(
                        out=gm_tiles[it][:, c0 * gpc : (c0 + w) * gpc],
                        in_=ps[:, : w * n_chunk].rearrange("p (a b) -> p a b", b=grp),
                        axis=X,
                        op=amin,
                    )
                    if c0 == last_c0:
                        s_sb = spool.tile([128, n_groups], fp32)
                        nc.vector.tensor_tensor(
                            out=s_sb[:], in0=gm_tiles[it][:], in1=nbg_sb, op=add
                        )
                        nc.vector.tensor_reduce(
                            out=m_sb[:, it : it + 1], in_=s_sb[:], axis=X, op=amin
                        )
            nc.sync.dma_start(out=mout[:], in_=m_sb[:])
    nc.compile()
    return nc


def prep_inputs(A_sel, B, row_tiles):
    """A_sel: [ROW_GROUPS*128*row_tiles, 512] f32 (padded), B: [M, 512] f32.
    Returns inb [N_CORES, 128, L] fp8 bytes (core = r*COL_GROUPS + g)."""
    e4 = ml_dtypes.float8_e4m3
    B32 = B.astype(np.float32)
    nb32 = (B32**2).sum(axis=1)
    # pad B with copies of column 0 (distance contributions duplicate, min unchanged)
    Bp = np.concatenate([B32, np.broadcast_to(B32[0:1], (M_PAD - M_B, D_FEAT))], axis=0)
    nbp = np.concatenate([nb32, np.broadcast_to(nb32[0:1], (M_PAD - M_B,))])
    order = np.argsort(nbp, kind="stable")
    Bs = Bp[order]
    nbs = nbp[order]

    n_per_rg = 128 * row_tiles
    AT = 512 * row_tiles
    L = AT + 4 * M_CORE + 4 * N_GROUPS

    # A region: [rg, 128p(feat%128), (tile, kt, half, 128i)] of -2A
    Am2 = (-2.0 * A_sel.astype(np.float32)).reshape(ROW_GROUPS, n_per_rg, D_FEAT)
    # feature index = kt*256 + half*128 + p
    atb = np.ascontiguousarray(
        Am2.reshape(ROW_GROUPS, row_tiles, 128, 2, 2, 128).transpose(0, 5, 1, 3, 4, 2)
    ).reshape(ROW_GROUPS, 128, AT).astype(e4)

    inb = np.empty((N_CORES, 128, L), np.uint8)
    for g in range(COL_GROUPS):
        Bg = Bs[g * M_CORE : (g + 1) * M_CORE]
        nbg_g = nbs[g * M_CORE : (g + 1) * M_CORE]
        # B region: [128p, kt(2), half(2), j] = Bg[j, kt*256+half*128+p]
        btb = np.ascontiguousarray(
            Bg.reshape(M_CORE, 2, 2, 128).transpose(3, 1, 2, 0)
        ).reshape(128, 4 * M_CORE).astype(e4)
        # per-group nb midpoint, broadcast to partitions, raw bytes
        gg = nbg_g.reshape(N_GROUPS, GRP)
        nb_mid = ((gg.min(axis=1) + gg.max(axis=1)) * 0.5).astype(np.float32)
        nbg_bytes = np.broadcast_to(
            nb_mid[None, :], (128, N_GROUPS)
        ).astype(np.float32).copy().view(np.uint8).reshape(128, 4 * N_GROUPS)
        for r in range(ROW_GROUPS):
            core = r * COL_GROUPS + g
            inb[core, :, 0:AT] = atb[r].view(np.uint8)
            inb[core, :, AT : AT + 4 * M_CORE] = btb.view(np.uint8)
            inb[core, :, AT + 4 * M_CORE : L] = nbg_bytes
    return inb.view(e4)


def _host_filter(A, B):
    """Sound row prefilter. Returns (survivor_indices, v0).

    ub_i = na_i + min_{j in S}(nb_j - 2 a_i.b_j) + pad >= d^2_min(i) for any
    probe subset S; pad absorbs fp32 matmul rounding. v0 = exact float64
    d^2_min of the best-ub row <= the true max. Rows with ub_i < v0 cannot
    be the argmax.
    """
    na = (A.astype(np.float64) ** 2).sum(axis=1)
    rng = np.random.default_rng(0)
    sel = rng.choice(B.shape[0], N_PROBE, replace=False)
    Bs = np.ascontiguousarray(B[sel]).astype(np.float32)
    nbs = (Bs.astype(np.float64) ** 2).sum(axis=1).astype(np.float32)
    G = np.ascontiguousarray(A.astype(np.float32)) @ Bs.T
    term = (nbs[None, :] - 2.0 * G).min(axis=1).astype(np.float64)
    ub = na + term + 1.0
    k = int(np.argmax(ub))
    B64 = B.astype(np.float64)
    d2k = na[k] + ((B64**2).sum(axis=1) - 2.0 * (B64 @ A[k].astype(np.float64)))
    v0 = float(d2k.min())
    surv = np.where(ub >= v0)[0]
    return surv, v0


def _exact_rescore(A, B, cand):
    A64 = A[cand].astype(np.float64)
    B64 = B.astype(np.float64)
    na = (A64 * A64).sum(axis=1)[:, None]
    nb = (B64 * B64).sum(axis=1)[None, :]
    sq = na - 2.0 * (A64 @ B64.T) + nb
    d = np.sqrt(np.maximum(sq, 0.0))
    return d.min(axis=1)


def _get_compiled(row_tiles):
    if row_tiles not in _compiled:
        _compiled[row_tiles] = build_program(row_tiles)
    return _compiled[row_tiles]


def kernel(A, B, _trace=False):
    from concourse.bass_utils import run_bass_kernel_spmd

    A = np.asarray(A, np.float32)
    B = np.asarray(B, np.float32)

    surv, _v0 = _host_filter(A, B)
    R = len(surv)
    tile_opts = [1, 2, 4, 8]
    T = next((t for t in tile_opts if t * 128 * ROW_GROUPS >= R), None)

    if T is None:
        # Overflow fallback (should not happen for randn inputs): exact
        # host rescore of every survivor, no device pass.
        d_exact = _exact_rescore(A, B, surv)
        w = int(np.argmax(d_exact))
        out = (np.array(int(surv[w]), dtype=np.int32),
               np.array(float(d_exact[w]), dtype=np.float32))
        if _trace:
            return out, None
        return out

    n_rows = T * 128 * ROW_GROUPS
    surv_pad = np.concatenate([surv, np.full(n_rows - R, surv[0], dtype=surv.dtype)])
    A_sel = A[surv_pad]

    inb = prep_inputs(A_sel, B, T)
    nc = _get_compiled(T)
    in_maps = [{"INB": inb[c]} for c in range(N_CORES)]
    res = run_bass_kernel_spmd(nc, in_maps, list(range(N_CORES)), trace=_trace)

    # Gather per-core m: core (r,g) -> [128, T]; combine min over g, then
    # undo the [128, T] (p, it) layout -> row it*128+p within row-group r.
    m_parts = []
    for r in range(ROW_GROUPS):
        mg = np.stack(
            [res.results[r * COL_GROUPS + g]["M"] for g in range(COL_GROUPS)]
        ).min(axis=0)  # [128, T]
        m_parts.append(mg.T.reshape(-1))
    m = np.concatenate(m_parts)
    na = (A_sel.astype(np.float64) ** 2).sum(axis=1)
    d_approx = np.sqrt(np.maximum(na + m, 0.0))
    v = d_approx.max()
    cand_local = np.where(d_approx >= v - DELTA)[0]
    cand = np.unique(surv_pad[cand_local])
    d_exact = _exact_rescore(A, B, cand)
    w = int(np.argmax(d_exact))
    idx = int(cand[w])
    val = float(d_exact[w])
    out = (np.array(idx, dtype=np.int32), np.array(val, dtype=np.float32))
    if _trace:
        return out, res
    return out


# revision 38
# speedup vs baseline: 1.0924x; 1.0902x over previous
"""K-center style kernel: argmax_i min_j ||A_i - B_j|| on 8 NeuronCores.

Strategy:
  - Host prefilter (sound): ub_i = na_i + min_{j in S}(nb_j - 2 a_i.b_j) + pad
    is a true upper bound on d^2_min(i) for any probe subset S (|S|=512).
    v0 = exact d^2_min (float64) of the best-ub row is a lower bound on the
    final max. Rows with ub_i < v0 cannot be the argmax and are dropped
    before touching the device (~98% of rows on randn inputs).
  - Hybrid shard over 8 cores: 4 row-groups x 2 column-groups. Core (r,g)
    gets survivor quarter r (padded to 128*T rows) and half g of B's
    columns (2560, sorted by nb = ||b||^2, B padded to 5120). Host takes
    the min over g and combines.
  - Per core: fp8 DoubleRow matmuls produce -2 a_i.b_j in PSUM; grouped
    DVE tensor_reduce(min) per PSUM tile gives per-group minima (groups of
    128 nb-sorted columns, nb midpoint added afterwards); per row-tile a
    tiny add(nb_mid) + min-reduce yields m[i] ~= min_j (nb_j - 2 a_i.b_j).
  - All device input (A-tiles, B^T, nb group mids as raw bytes) is packed
    into one fp8 DRAM tensor so every DMA has >=2KB-per-partition
    descriptors, split across the sync+scalar HWDGE queues.
  - Host: D_approx = sqrt(max(na + m, 0)) over survivors; select candidates
    within DELTA of the max; rescore candidates exactly in float64; return
    (argmax int32, max float32).

The host rescore makes the final answer exact regardless of device
precision; the device pass only needs the true argmax inside the
candidate set. Device error sources: fp8 input rounding + nb grouping,
both ~1e-2 in D units. DELTA = 1.0 is far above both.
"""

import numpy as np
import ml_dtypes

N_CORES = 8
ROW_GROUPS = 4
COL_GROUPS = 2
M_B = 5000
M_PAD = 5120                              # padded B columns (sorted by nb)
M_CORE = M_PAD // COL_GROUPS              # 2560 columns per core
D_FEAT = 512
N_CHUNK = 512                             # matmul free dim = one fp32 PSUM bank
GRP = 128                                 # B columns per min-group (sorted by nb)
N_GROUPS = M_CORE // GRP                  # 20
N_PROBE = 512                             # host prefilter probe columns

DELTA = 1.0  # candidate slack in D units (covers fp8 e4m3 + grouping error)

_compiled = {}


def build_program(row_tiles, m_b=M_CORE, n_chunk=N_CHUNK, grp=GRP):
    import concourse.tile as tile
    import concourse.mybir as mybir
    from concourse import bacc

    n_chunks = m_b // n_chunk            # 5
    n_groups = m_b // grp                # 20
    gpc = n_chunk // grp                 # groups per chunk
    assert m_b % n_chunk == 0 and n_chunk % grp == 0

    AT = 512 * row_tiles                 # A region bytes per partition
    L = AT + 4 * m_b + 4 * n_groups      # + B^T region + nbg raw bytes

    nc = bacc.Bacc("TRN2", target_bir_lowering=False, debug=False)
    inb = nc.dram_tensor(
        "INB", [128, L], mybir.dt.float8e4, kind="ExternalInput"
    ).ap()
    mout = nc.dram_tensor(
        "M", [128, row_tiles], mybir.dt.float32, kind="ExternalOutput"
    ).ap()

    fp32 = mybir.dt.float32
    fp8 = mybir.dt.float8e4
    DR = mybir.MatmulPerfMode.DoubleRow
    add = mybir.AluOpType.add
    amin = mybir.AluOpType.min
    X = mybir.AxisListType.X

    # chunk groups -> one PSUM tile each; last group small so the final
    # DVE reduce (tail latency after the last matmul) is short
    psgroups = []
    c = 0
    while c < n_chunks:
        w = min(4, n_chunks - c)
        psgroups.append((c, w))
        c += w

    def bofs(kt, half, c0):
        return AT + kt * 2 * m_b + half * m_b + c0 * n_chunk

    with tile.TileContext(nc) as tc:
        with (
            tc.tile_pool(name="const", bufs=1) as cpool,
            tc.tile_pool(name="psum", bufs=2, space="PSUM") as pspool,
            tc.tile_pool(name="work", bufs=2 * row_tiles + 2) as gmpool,
        ):
            spool = mpool = gmpool
            # PE p-state warmup: the PE only reaches full clock after ~3us
            # of continuous execution, and it would otherwise sit idle while
            # the input DMAs land. Stream dummy matmuls (memset SBUF ->
            # sacrificial PSUM bank) from t~6.5us so the real matmuls issue
            # against a hot array.
            dummy_sb = cpool.tile([128, 1024], fp8)
            nc.gpsimd.memset(dummy_sb[:], 0)
            first_ps = pspool.tile([128, 4 * n_chunk], fp32, tag="ps")
            for _ in range(6):
                nc.tensor.matmul(
                    first_ps[:, 0:n_chunk],
                    lhsT=dummy_sb[:, 0:256].rearrange("p (two f) -> p two f", two=2),
                    rhs=dummy_sb[:].rearrange("p (two j) -> p two j", two=2),
                    start=True,
                    stop=True,
                    perf_mode=DR,
                )

            # One resident SBUF tile holds A-tiles | B^T | nbg bytes, loaded
            # by 8 DMAs (2KB+ descriptors) alternating sync/scalar queues,
            # first-psum-group slices first so matmuls unblock earliest.
            # Group-0 B slices go in 2-chunk halves per (kt, half) so the
            # kt1 accumulate passes unblock ~2us earlier; h0 on sync, h1 on
            # scalar. The first sync slice also carries the A region.
            inb_sb = cpool.tile([128, L], fp8)
            c0, w0 = psgroups[0]
            for cp in range(0, w0, 2):
                cw = min(2, w0 - cp)
                for kt in range(2):
                    for half in range(2):
                        lo = bofs(kt, half, cp)
                        hi = lo + cw * n_chunk
                        eng = nc.sync if half == 0 else nc.scalar
                        if cp == 0 and kt == 0 and half == 0:
                            eng.dma_start(out=inb_sb[:, 0:hi], in_=inb[:, 0:hi])
                        else:
                            eng.dma_start(out=inb_sb[:, lo:hi], in_=inb[:, lo:hi])
            qflip = 0
            for c1, w1 in psgroups[1:]:
                for kt in range(2):
                    for half in range(2):
                        lo = bofs(kt, half, c1)
                        hi = lo + w1 * n_chunk
                        if kt == 1 and half == 1 and (c1, w1) == psgroups[-1]:
                            hi = L  # append nbg bytes to the last B slice
                        eng = nc.sync if qflip % 2 == 0 else nc.scalar
                        qflip += 1
                        eng.dma_start(out=inb_sb[:, lo:hi], in_=inb[:, lo:hi])

            a_all = inb_sb[:, 0:AT]
            nbg_sb = inb_sb[:, AT + 4 * m_b : L].bitcast(fp32)  # [128, n_groups]
            m_sb = mpool.tile([128, row_tiles], fp32)

            gm_tiles = [
                gmpool.tile([128, n_groups], fp32, tag="gm", name=f"gm{i}")
                for i in range(row_tiles)
            ]
            last_c0 = psgroups[-1][0]
            for c0, w in psgroups:
                for it in range(row_tiles):
                    if c0 == 0 and it == 0:
                        ps = first_ps
                    else:
                        ps = pspool.tile([128, 4 * n_chunk], fp32, tag="ps")
                    for nl in range(w):
                        n = c0 + nl
                        for kt in range(2):
                            lhsT3 = a_all[
                                :, it * 512 + kt * 256 : it * 512 + (kt + 1) * 256
                            ].rearrange("p (two f) -> p two f", two=2)
                            # rhs: [p, two(half), j] strided view over inb_sb
                            rhs = inb_sb[:, bofs(kt, 0, 0) : bofs(kt, 0, 0) + 2 * m_b]
                            rhs = rhs.rearrange("p (two j) -> p two j", two=2)
                            nc.tensor.matmul(
                                ps[:, nl * n_chunk : (nl + 1) * n_chunk],
                                lhsT=lhsT3,
                                rhs=rhs[:, :, n * n_chunk : (n + 1) * n_chunk],
                                start=(kt == 0),
                                stop=(kt == 1),
                                perf_mode=DR,
                            )
                    nc.vector.tensor_reduce(
                        out=gm_tiles[it][:, c0 * gpc : (c0 + w) * gpc],
                        in_=ps[:, : w * n_chunk].rearrange("p (a b) -> p a b", b=grp),
                        axis=X,
                        op=amin,
                    )
                    if c0 == last_c0:
                        s_sb = spool.tile([128, n_groups], fp32)
                        nc.vector.tensor_tensor(
                            out=s_sb[:], in0=gm_tiles[it][:], in1=nbg_sb, op=add
                        )
                        nc.vector.tensor_reduce(
                            out=m_sb[:, it : it + 1], in_=s_sb[:], axis=X, op=amin
                        )
            nc.sync.dma_start(out=mout[:], in_=m_sb[:])
    nc.compile()
    return nc


def prep_inputs(A_sel, B, row_tiles):
    """A_sel: [ROW_GROUPS*128*row_tiles, 512] f32 (padded), B: [M, 512] f32.
    Returns inb [N_CORES, 128, L] fp8 bytes (core = r*COL_GROUPS + g)."""
    e4 = ml_dtypes.float8_e4m3
    B32 = B.astype(np.float32)
    nb32 = (B32**2).sum(axis=1)
    # pad B with copies of column 0 (distance contributions duplicate, min unchanged)
    Bp = np.concatenate([B32, np.broadcast_to(B32[0:1], (M_PAD - M_B, D_FEAT))], axis=0)
    nbp = np.concatenate([nb32, np.broadcast_to(nb32[0:1], (M_PAD - M_B,))])
    order = np.argsort(nbp, kind="stable")
    Bs = Bp[order]
    nbs = nbp[order]

    n_per_rg = 128 * row_tiles
    AT = 512 * row_tiles
    L = AT + 4 * M_CORE + 4 * N_GROUPS

    # A region: [rg, 128p(feat%128), (tile, kt, half, 128i)] of -2A
    Am2 = (-2.0 * A_sel.astype(np.float32)).reshape(ROW_GROUPS, n_per_rg, D_FEAT)
    # feature index = kt*256 + half*128 + p
    atb = np.ascontiguousarray(
        Am2.reshape(ROW_GROUPS, row_tiles, 128, 2, 2, 128).transpose(0, 5, 1, 3, 4, 2)
    ).reshape(ROW_GROUPS, 128, AT).astype(e4)

    inb = np.empty((N_CORES, 128, L), np.uint8)
    for g in range(COL_GROUPS):
        Bg = Bs[g * M_CORE : (g + 1) * M_CORE]
        nbg_g = nbs[g * M_CORE : (g + 1) * M_CORE]
        # B region: [128p, kt(2), half(2), j] = Bg[j, kt*256+half*128+p]
        btb = np.ascontiguousarray(
            Bg.reshape(M_CORE, 2, 2, 128).transpose(3, 1, 2, 0)
        ).reshape(128, 4 * M_CORE).astype(e4)
        # per-group nb midpoint, broadcast to partitions, raw bytes
        gg = nbg_g.reshape(N_GROUPS, GRP)
        nb_mid = ((gg.min(axis=1) + gg.max(axis=1)) * 0.5).astype(np.float32)
        nbg_bytes = np.broadcast_to(
            nb_mid[None, :], (128, N_GROUPS)
        ).astype(np.float32).copy().view(np.uint8).reshape(128, 4 * N_GROUPS)
        for r in range(ROW_GROUPS):
            core = r * COL_GROUPS + g
            inb[core, :, 0:AT] = atb[r].view(np.uint8)
            inb[core, :, AT : AT + 4 * M_CORE] = btb.view(np.uint8)
            inb[core, :, AT + 4 * M_CORE : L] = nbg_bytes
    return inb.view(e4)


def _host_filter(A, B):
    """Sound row prefilter. Returns (survivor_indices, v0).

    ub_i = na_i + min_{j in S}(nb_j - 2 a_i.b_j) + pad >= d^2_min(i) for any
    probe subset S; pad absorbs fp32 matmul rounding. v0 = exact float64
    d^2_min of the best-ub row <= the true max. Rows with ub_i < v0 cannot
    be the argmax.
    """
    na = (A.astype(np.float64) ** 2).sum(axis=1)
    rng = np.random.default_rng(0)
    sel = rng.choice(B.shape[0], N_PROBE, replace=False)
    Bs = np.ascontiguousarray(B[sel]).astype(np.float32)
    nbs = (Bs.astype(np.float64) ** 2).sum(axis=1).astype(np.float32)
    G = np.ascontiguousarray(A.astype(np.float32)) @ Bs.T
    term = (nbs[None, :] - 2.0 * G).min(axis=1).astype(np.float64)
    ub = na + term + 1.0
    k = int(np.argmax(ub))
    B64 = B.astype(np.float64)
    d2k = na[k] + ((B64**2).sum(axis=1) - 2.0 * (B64 @ A[k].astype(np.float64)))
    v0 = float(d2k.min())
    surv = np.where(ub >= v0)[0]
    return surv, v0


def _exact_rescore(A, B, cand):
    A64 = A[cand].astype(np.float64)
    B64 = B.astype(np.float64)
    na = (A64 * A64).sum(axis=1)[:, None]
    nb = (B64 * B64).sum(axis=1)[None, :]
    sq = na - 2.0 * (A64 @ B64.T) + nb
    d = np.sqrt(np.maximum(sq, 0.0))
    return d.min(axis=1)


def _get_compiled(row_tiles):
    if row_tiles not in _compiled:
        _compiled[row_tiles] = build_program(row_tiles)
    return _compiled[row_tiles]


def kernel(A, B, _trace=False):
    from concourse.bass_utils import run_bass_kernel_spmd

    A = np.asarray(A, np.float32)
    B = np.asarray(B, np.float32)

    surv, _v0 = _host_filter(A, B)
    R = len(surv)
    tile_opts = [1, 2, 4, 8]
    T = next((t for t in tile_opts if t * 128 * ROW_GROUPS >= R), None)

    if T is None:
        # Overflow fallback (should not happen for randn inputs): exact
        # host rescore of every survivor, no device pass.
        d_exact = _exact_rescore(A, B, surv)
        w = int(np.argmax(d_exact))
        out = (np.array(int(surv[w]), dtype=np.int32),
               np.array(float(d_exact[w]), dtype=np.float32))
        if _trace:
            return out, None
        return out

    n_rows = T * 128 * ROW_GROUPS
    surv_pad = np.concatenate([surv, np.full(n_rows - R, surv[0], dtype=surv.dtype)])
    A_sel = A[surv_pad]

    inb = prep_inputs(A_sel, B, T)
    nc = _get_compiled(T)
    in_maps = [{"INB": inb[c]} for c in range(N_CORES)]
    res = run_bass_kernel_spmd(nc, in_maps, list(range(N_CORES)), trace=_trace)

    # Gather per-core m: core (r,g) -> [128, T]; combine min over g, then
    # undo the [128, T] (p, it) layout -> row it*128+p within row-group r.
    m_parts = []
    for r in range(ROW_GROUPS):
        mg = np.stack(
            [res.results[r * COL_GROUPS + g]["M"] for g in range(COL_GROUPS)]
        ).min(axis=0)  # [128, T]
        m_parts.append(mg.T.reshape(-1))
    m = np.concatenate(m_parts)
    na = (A_sel.astype(np.float64) ** 2).sum(axis=1)
    d_approx = np.sqrt(np.maximum(na + m, 0.0))
    v = d_approx.max()
    cand_local = np.where(d_approx >= v - DELTA)[0]
    cand = np.unique(surv_pad[cand_local])
    d_exact = _exact_rescore(A, B, cand)
    w = int(np.argmax(d_exact))
    idx = int(cand[w])
    val = float(d_exact[w])
    out = (np.array(idx, dtype=np.int32), np.array(val, dtype=np.float32))
    if _trace:
        return out, res
    return out


# revision 39
# speedup vs baseline: 1.1326x; 1.0368x over previous
"""K-center style kernel: argmax_i min_j ||A_i - B_j|| on 8 NeuronCores.

Strategy:
  - Host prefilter (sound): ub_i = na_i + min_{j in S}(nb_j - 2 a_i.b_j) + pad
    is a true upper bound on d^2_min(i) for any probe subset S (|S|=512).
    v0 = exact d^2_min (float64) of the best-ub row is a lower bound on the
    final max. Rows with ub_i < v0 cannot be the argmax and are dropped
    before touching the device (~98% of rows on randn inputs).
  - Hybrid shard over 8 cores: 4 row-groups x 2 column-groups. Core (r,g)
    gets survivor quarter r (padded to 128*T rows) and half g of B's
    columns (2560, sorted by nb = ||b||^2, B padded to 5120). Host takes
    the min over g and combines.
  - Per core: fp8 DoubleRow matmuls produce -2 a_i.b_j in PSUM; grouped
    DVE tensor_reduce(min) per PSUM tile gives per-group minima (groups of
    128 nb-sorted columns, nb midpoint added afterwards); per row-tile a
    tiny add(nb_mid) + min-reduce yields m[i] ~= min_j (nb_j - 2 a_i.b_j).
  - All device input (A-tiles, B^T, nb group mids as raw bytes) is packed
    into one fp8 DRAM tensor so every DMA has >=2KB-per-partition
    descriptors, split across the sync+scalar HWDGE queues.
  - Host: D_approx = sqrt(max(na + m, 0)) over survivors; select candidates
    within DELTA of the max; rescore candidates exactly in float64; return
    (argmax int32, max float32).

The host rescore makes the final answer exact regardless of device
precision; the device pass only needs the true argmax inside the
candidate set. Device error sources: fp8 input rounding + nb grouping,
both ~1e-2 in D units. DELTA = 1.0 is far above both.
"""

import numpy as np
import ml_dtypes

N_CORES = 8
ROW_GROUPS = 4
COL_GROUPS = 2
M_B = 5000
M_PAD = 5120                              # padded B columns (sorted by nb)
M_CORE = M_PAD // COL_GROUPS              # 2560 columns per core
D_FEAT = 512
N_CHUNK = 512                             # matmul free dim = one fp32 PSUM bank
GRP = 128                                 # B columns per min-group (sorted by nb)
N_GROUPS = M_CORE // GRP                  # 20
N_PROBE = 512                             # host prefilter probe columns

DELTA = 1.0  # candidate slack in D units (covers fp8 e4m3 + grouping error)

_compiled = {}


def build_program(row_tiles, m_b=M_CORE, n_chunk=N_CHUNK, grp=GRP):
    import concourse.tile as tile
    import concourse.mybir as mybir
    from concourse import bacc

    n_chunks = m_b // n_chunk            # 5
    n_groups = m_b // grp                # 20
    gpc = n_chunk // grp                 # groups per chunk
    assert m_b % n_chunk == 0 and n_chunk % grp == 0

    AT = 512 * row_tiles                 # A region bytes per partition
    L = AT + 4 * m_b + 4 * n_groups      # + B^T region + nbg raw bytes

    nc = bacc.Bacc("TRN2", target_bir_lowering=False, debug=False)
    inb = nc.dram_tensor(
        "INB", [128, L], mybir.dt.float8e4, kind="ExternalInput"
    ).ap()
    mout = nc.dram_tensor(
        "M", [128, row_tiles], mybir.dt.float32, kind="ExternalOutput"
    ).ap()

    fp32 = mybir.dt.float32
    fp8 = mybir.dt.float8e4
    DR = mybir.MatmulPerfMode.DoubleRow
    add = mybir.AluOpType.add
    amin = mybir.AluOpType.min
    X = mybir.AxisListType.X

    # chunk groups -> one PSUM tile each; last group small so the final
    # DVE reduce (tail latency after the last matmul) is short
    psgroups = []
    c = 0
    while c < n_chunks:
        w = min(4, n_chunks - c)
        psgroups.append((c, w))
        c += w

    def bofs(kt, half, c0):
        return AT + kt * 2 * m_b + half * m_b + c0 * n_chunk

    with tile.TileContext(nc) as tc:
        with (
            tc.tile_pool(name="const", bufs=1) as cpool,
            tc.tile_pool(name="psum", bufs=2, space="PSUM") as pspool,
            tc.tile_pool(name="work", bufs=2 * row_tiles + 2) as gmpool,
        ):
            spool = mpool = gmpool
            # PE p-state warmup: the PE only reaches full clock after ~3us
            # of continuous execution, and it would otherwise sit idle while
            # the input DMAs land. Stream dummy matmuls (memset SBUF ->
            # sacrificial PSUM bank) from t~6.5us so the real matmuls issue
            # against a hot array.
            dummy_sb = cpool.tile([128, 1024], fp8)
            nc.gpsimd.memset(dummy_sb[:], 0)
            first_ps = pspool.tile([128, 4 * n_chunk], fp32, tag="ps")
            for _ in range(9):
                nc.tensor.matmul(
                    first_ps[:, 0:n_chunk],
                    lhsT=dummy_sb[:, 0:256].rearrange("p (two f) -> p two f", two=2),
                    rhs=dummy_sb[:].rearrange("p (two j) -> p two j", two=2),
                    start=True,
                    stop=True,
                    perf_mode=DR,
                )

            # One resident SBUF tile holds A-tiles | B^T | nbg bytes, loaded
            # by 8 DMAs (2KB+ descriptors) alternating sync/scalar queues,
            # first-psum-group slices first so matmuls unblock earliest.
            # Group-0 B slices go in 2-chunk halves per (kt, half) so the
            # kt1 accumulate passes unblock ~2us earlier; h0 on sync, h1 on
            # scalar. The first sync slice also carries the A region.
            inb_sb = cpool.tile([128, L], fp8)
            c0, w0 = psgroups[0]
            for cp in range(0, w0, 2):
                cw = min(2, w0 - cp)
                for kt in range(2):
                    for half in range(2):
                        lo = bofs(kt, half, cp)
                        hi = lo + cw * n_chunk
                        eng = nc.sync if half == 0 else nc.scalar
                        if cp == 0 and kt == 0 and half == 0:
                            eng.dma_start(out=inb_sb[:, 0:hi], in_=inb[:, 0:hi])
                        else:
                            eng.dma_start(out=inb_sb[:, lo:hi], in_=inb[:, lo:hi])
            qflip = 0
            for c1, w1 in psgroups[1:]:
                for kt in range(2):
                    for half in range(2):
                        lo = bofs(kt, half, c1)
                        hi = lo + w1 * n_chunk
                        if kt == 1 and half == 1 and (c1, w1) == psgroups[-1]:
                            hi = L  # append nbg bytes to the last B slice
                        eng = nc.sync if qflip % 2 == 0 else nc.scalar
                        qflip += 1
                        eng.dma_start(out=inb_sb[:, lo:hi], in_=inb[:, lo:hi])

            a_all = inb_sb[:, 0:AT]
            nbg_sb = inb_sb[:, AT + 4 * m_b : L].bitcast(fp32)  # [128, n_groups]
            m_sb = mpool.tile([128, row_tiles], fp32)

            gm_tiles = [
                gmpool.tile([128, n_groups], fp32, tag="gm", name=f"gm{i}")
                for i in range(row_tiles)
            ]
            last_c0 = psgroups[-1][0]
            for c0, w in psgroups:
                for it in range(row_tiles):
                    if c0 == 0 and it == 0:
                        ps = first_ps
                    else:
                        ps = pspool.tile([128, 4 * n_chunk], fp32, tag="ps")
                    for nl in range(w):
                        n = c0 + nl
                        for kt in range(2):
                            lhsT3 = a_all[
                                :, it * 512 + kt * 256 : it * 512 + (kt + 1) * 256
                            ].rearrange("p (two f) -> p two f", two=2)
                            # rhs: [p, two(half), j] strided view over inb_sb
                            rhs = inb_sb[:, bofs(kt, 0, 0) : bofs(kt, 0, 0) + 2 * m_b]
                            rhs = rhs.rearrange("p (two j) -> p two j", two=2)
                            nc.tensor.matmul(
                                ps[:, nl * n_chunk : (nl + 1) * n_chunk],
                                lhsT=lhsT3,
                                rhs=rhs[:, :, n * n_chunk : (n + 1) * n_chunk],
                                start=(kt == 0),
                                stop=(kt == 1),
                                perf_mode=DR,
                            )
                    nc.vector.tensor_reduce(
                        out=gm_tiles[it][:, c0 * gpc : (c0 + w) * gpc],
                        in_=ps[:, : w * n_chunk].rearrange("p (a b) -> p a b", b=grp),
                        axis=X,
                        op=amin,
                    )
                    if c0 == last_c0:
                        s_sb = spool.tile([128, n_groups], fp32)
                        nc.vector.tensor_tensor(
                            out=s_sb[:], in0=gm_tiles[it][:], in1=nbg_sb, op=add
                        )
                        nc.vector.tensor_reduce(
                            out=m_sb[:, it : it + 1], in_=s_sb[:], axis=X, op=amin
                        )
            nc.sync.dma_start(out=mout[:], in_=m_sb[:])
    nc.compile()
    return nc


def prep_inputs(A_sel, B, row_tiles):
    """A_sel: [ROW_GROUPS*128*row_tiles, 512] f32 (padded), B: [M, 512] f32.
    Returns inb [N_CORES, 128, L] fp8 bytes (core = r*COL_GROUPS + g)."""
    e4 = ml_dtypes.float8_e4m3
    B32 = B.astype(np.float32)
    nb32 = (B32**2).sum(axis=1)
    # pad B with copies of column 0 (distance contributions duplicate, min unchanged)
    Bp = np.concatenate([B32, np.broadcast_to(B32[0:1], (M_PAD - M_B, D_FEAT))], axis=0)
    nbp = np.concatenate([nb32, np.broadcast_to(nb32[0:1], (M_PAD - M_B,))])
    order = np.argsort(nbp, kind="stable")
    Bs = Bp[order]
    nbs = nbp[order]

    n_per_rg = 128 * row_tiles
    AT = 512 * row_tiles
    L = AT + 4 * M_CORE + 4 * N_GROUPS

    # A region: [rg, 128p(feat%128), (tile, kt, half, 128i)] of -2A
    Am2 = (-2.0 * A_sel.astype(np.float32)).reshape(ROW_GROUPS, n_per_rg, D_FEAT)
    # feature index = kt*256 + half*128 + p
    atb = np.ascontiguousarray(
        Am2.reshape(ROW_GROUPS, row_tiles, 128, 2, 2, 128).transpose(0, 5, 1, 3, 4, 2)
    ).reshape(ROW_GROUPS, 128, AT).astype(e4)

    inb = np.empty((N_CORES, 128, L), np.uint8)
    for g in range(COL_GROUPS):
        Bg = Bs[g * M_CORE : (g + 1) * M_CORE]
        nbg_g = nbs[g * M_CORE : (g + 1) * M_CORE]
        # B region: [128p, kt(2), half(2), j] = Bg[j, kt*256+half*128+p]
        btb = np.ascontiguousarray(
            Bg.reshape(M_CORE, 2, 2, 128).transpose(3, 1, 2, 0)
        ).reshape(128, 4 * M_CORE).astype(e4)
        # per-group nb midpoint, broadcast to partitions, raw bytes
        gg = nbg_g.reshape(N_GROUPS, GRP)
        nb_mid = ((gg.min(axis=1) + gg.max(axis=1)) * 0.5).astype(np.float32)
        nbg_bytes = np.broadcast_to(
            nb_mid[None, :], (128, N_GROUPS)
        ).astype(np.float32).copy().view(np.uint8).reshape(128, 4 * N_GROUPS)
        for r in range(ROW_GROUPS):
            core = r * COL_GROUPS + g
            inb[core, :, 0:AT] = atb[r].view(np.uint8)
            inb[core, :, AT : AT + 4 * M_CORE] = btb.view(np.uint8)
            inb[core, :, AT + 4 * M_CORE : L] = nbg_bytes
    return inb.view(e4)


def _host_filter(A, B):
    """Sound row prefilter. Returns (survivor_indices, v0).

    ub_i = na_i + min_{j in S}(nb_j - 2 a_i.b_j) + pad >= d^2_min(i) for any
    probe subset S; pad absorbs fp32 matmul rounding. v0 = exact float64
    d^2_min of the best-ub row <= the true max. Rows with ub_i < v0 cannot
    be the argmax.
    """
    na = (A.astype(np.float64) ** 2).sum(axis=1)
    rng = np.random.default_rng(0)
    sel = rng.choice(B.shape[0], N_PROBE, replace=False)
    Bs = np.ascontiguousarray(B[sel]).astype(np.float32)
    nbs = (Bs.astype(np.float64) ** 2).sum(axis=1).astype(np.float32)
    G = np.ascontiguousarray(A.astype(np.float32)) @ Bs.T
    term = (nbs[None, :] - 2.0 * G).min(axis=1).astype(np.float64)
    ub = na + term + 1.0
    k = int(np.argmax(ub))
    B64 = B.astype(np.float64)
    d2k = na[k] + ((B64**2).sum(axis=1) - 2.0 * (B64 @ A[k].astype(np.float64)))
    v0 = float(d2k.min())
    surv = np.where(ub >= v0)[0]
    return surv, v0


def _exact_rescore(A, B, cand):
    A64 = A[cand].astype(np.float64)
    B64 = B.astype(np.float64)
    na = (A64 * A64).sum(axis=1)[:, None]
    nb = (B64 * B64).sum(axis=1)[None, :]
    sq = na - 2.0 * (A64 @ B64.T) + nb
    d = np.sqrt(np.maximum(sq, 0.0))
    return d.min(axis=1)


def _get_compiled(row_tiles):
    if row_tiles not in _compiled:
        _compiled[row_tiles] = build_program(row_tiles)
    return _compiled[row_tiles]


def kernel(A, B, _trace=False):
    from concourse.bass_utils import run_bass_kernel_spmd

    A = np.asarray(A, np.float32)
    B = np.asarray(B, np.float32)

    surv, _v0 = _host_filter(A, B)
    R = len(surv)
    tile_opts = [1, 2, 4, 8]
    T = next((t for t in tile_opts if t * 128 * ROW_GROUPS >= R), None)

    if T is None:
        # Overflow fallback (should not happen for randn inputs): exact
        # host rescore of every survivor, no device pass.
        d_exact = _exact_rescore(A, B, surv)
        w = int(np.argmax(d_exact))
        out = (np.array(int(surv[w]), dtype=np.int32),
               np.array(float(d_exact[w]), dtype=np.float32))
        if _trace:
            return out, None
        return out

    n_rows = T * 128 * ROW_GROUPS
    surv_pad = np.concatenate([surv, np.full(n_rows - R, surv[0], dtype=surv.dtype)])
    A_sel = A[surv_pad]

    inb = prep_inputs(A_sel, B, T)
    nc = _get_compiled(T)
    in_maps = [{"INB": inb[c]} for c in range(N_CORES)]
    res = run_bass_kernel_spmd(nc, in_maps, list(range(N_CORES)), trace=_trace)

    # Gather per-core m: core (r,g) -> [128, T]; combine min over g, then
    # undo the [128, T] (p, it) layout -> row it*128+p within row-group r.
    m_parts = []
    for r in range(ROW_GROUPS):
        mg = np.stack(
            [res.results[r * COL_GROUPS + g]["M"] for g in range(COL_GROUPS)]
        ).min(axis=0)  # [128, T]
        m_parts.append(mg.T.reshape(-1))
    m = np.concatenate(m_parts)
    na = (A_sel.astype(np.float64) ** 2).sum(axis=1)
    d_approx = np.sqrt(np.maximum(na + m, 0.0))
    v = d_approx.max()
    cand_local = np.where(d_approx >= v - DELTA)[0]
    cand = np.unique(surv_pad[cand_local])
    d_exact = _exact_rescore(A, B, cand)
    w = int(np.argmax(d_exact))
    idx = int(cand[w])
    val = float(d_exact[w])
    out = (np.array(idx, dtype=np.int32), np.array(val, dtype=np.float32))
    if _trace:
        return out, res
    return out
